# revision 11
# baseline (speedup 1.0000x reference)
"""Trainium2 Bass kernel for nn_Expert (gather-span + 2-layer linear MLP).

Reference computation (B=32, L=4096, H=1024, N=4):
    idx      = pos + arange(N)                      # (B, N)
    gathered = hidden[b, idx[b, n], :]              # (B, N, H)
    x        = gathered.reshape(B, N*H)             # (B, 4096)
    out      = (x @ W1.T + b1) @ W2.T + b2          # (B, 4)

Sharding (8 cores): hidden sharded on the LAST dim (H) in 128-wide
slices; W1 sharded over the matching contraction columns (2MB fp32 per
core -- the dominant HBM traffic); pos replicated as an int32 column;
W2 replicated, packed as a 32-col tail on the W1 dram tensor; b1/b2
fold into the host-side constant c = W2 @ b1 + b2 (pure weight prep).

Design notes (from trace analysis):
  * exec time = span minus a ~6us fixed prologue, but the sem-reset
    teardown DOES count and costs ~50-90ns per allocated semaphore --
    keep the instruction/edge count small.
  * fp32 matmul runs at 4 cycles/row; the PE HAM clock gate starts at
    1.2 GHz and reaches 2.4 GHz only after ~3.4us of sustained PE work,
    so zero-dependency junk matmuls warm the array while W1 streams.
  * stage 1 is b-major (xT chunk stationary -- only 32-col LDWEIGHTS;
    W1 streams as the moving operand) in two 512-col PSUM banks; the
    four W1 chunk DMAs land left-to-right so matmuls chase the stream.
  * stage 2 PE-transposes out1 back to o-on-partitions and contracts
    with the packed W2 block: no wide DVE reductions, no replication.
"""

import numpy as np

from concourse import bass, bacc, mybir
from concourse.tile import TileContext
from concourse.bass_utils import run_bass_kernel_spmd
from concourse.masks import make_identity

B, L, H, N = 32, 4096, 1024, 4
NCORES = 8
HS = H // NCORES       # 128: per-core slice of the hidden dim
P = 128
NOT = 8                # o-tiles of 128 in stage 2
HB = 512               # psum bank width (fp32)
W1W = 1024             # free cols per n-chunk of w1
W2OFF = N * W1W        # 4096: w2 block offset in the combined tensor
F32 = mybir.dt.float32
I32 = mybir.dt.int32

WARMUP_MMS = 7         # junk fp32 N=128 matmuls to warm the PE clock

TRACE = False
LAST_EXEC_NS = None

_nc_cache = None


def _build_nc():
    nc = bacc.Bacc(target_bir_lowering=False)
    hid = nc.declare_dram_parameter("hid", [B * L, HS], F32, isOutput=False)
    posr = nc.declare_dram_parameter("posr", [B, 1], I32, isOutput=False)
    w1t = nc.declare_dram_parameter("w1t", [P, W2OFF + 32], F32, isOutput=False)
    out = nc.declare_dram_parameter("out", [N, B], F32, isOutput=True)

    with TileContext(nc) as tc:
        with (
            tc.tile_pool(name="sbuf", bufs=1) as spool,
            tc.tile_pool(name="psum", bufs=1, space="PSUM") as ppool,
        ):
            # ---- pos column first; the W1 stream is gated behind the
            # gather (below), so this completes on quiet queues
            pos_col = spool.tile([B, 1], I32)
            nc.gpsimd.dma_start(out=pos_col[:], in_=posr[:])
            w1sb = []
            for n in range(N):
                w = W1W if n < N - 1 else W1W + 32
                t = spool.tile([P, w], F32, tag=f"w1_{n}", name=f"w1_{n}")
                w1sb.append(t)

            # ---- junk matmuls warm the PE clock gate while W1 streams
            junk = spool.tile([P, P], F32, tag="junk")
            nc.vector.memset(junk[:], 0.0)
            wu_ps = ppool.tile([P, P], F32, space="PSUM", tag="wu")
            for i in range(WARMUP_MMS):
                nc.tensor.matmul(
                    out=wu_ps[:], lhsT=junk[:], rhs=junk[:],
                    start=True, stop=True,
                )

            # ---- gather indices: idx[b] = b*L + pos[b]
            gci = spool.tile([B, 1], I32)
            nc.gpsimd.iota(gci[:], pattern=[[0, 1]], base=0,
                           channel_multiplier=L)
            idx = spool.tile([B, 1], I32)
            nc.gpsimd.tensor_tensor(
                out=idx[:], in0=gci[:], in1=pos_col[:],
                op=mybir.AluOpType.add,
            )

            # ---- gather: 32 spans of 4 consecutive hid rows (2KB each)
            xg = spool.tile([B, N * HS], F32)
            nc.gpsimd.indirect_dma_start(
                out=xg[:, :],
                out_offset=None,
                in_=hid[:],
                in_offset=bass.IndirectOffsetOnAxis(ap=idx[:, :1], axis=0),
                bounds_check=B * L - 1,
                oob_is_err=False,
            )

            # ---- W1 chunks, gated behind xg: a stub write into each chunk
            # tile that reads xg forces the chunk DMA (WAR) to wait until
            # the gather has drained -- tiny DMAs starve for ~3.5us when
            # the 2MB W1 stream floods the 16 SDMA engines first.
            for n in range(N):
                nc.vector.tensor_copy(out=w1sb[n][:1, :1], in_=xg[:1, :1])
                nc.scalar.dma_start(
                    out=w1sb[n][:],
                    in_=w1t[:, n * W1W:n * W1W + (W1W if n < N - 1 else W1W + 32)],
                )

            # ---- transpose each 128-col chunk: xT[:, n*32+b] = x[b,(n,hp)]
            ident = spool.tile([P, P], F32)
            make_identity(nc, ident[:])
            xT_ps = ppool.tile([P, P], F32, space="PSUM", tag="xt")
            xT = spool.tile([P, P], F32)
            for n in range(N):
                nc.tensor.transpose(
                    out=xT_ps[:, n * B:(n + 1) * B],
                    in_=xg[:, n * HS:(n + 1) * HS],
                    identity=ident[:B, :B],
                )
            nc.vector.tensor_copy(out=xT[:], in_=xT_ps[:])

            # ---- stage 1 b-major: out1[b, o] in two 512-col banks
            ps_h = [
                ppool.tile([B, HB], F32, space="PSUM", tag=f"ps{h}",
                           name=f"ps{h}")
                for h in range(2)
            ]
            for n in range(N):
                for h in range(2):
                    nc.tensor.matmul(
                        out=ps_h[h][:],
                        lhsT=xT[:, n * B:(n + 1) * B],
                        rhs=w1sb[n][:, h * HB:(h + 1) * HB],
                        start=(n == 0),
                        stop=(n == N - 1),
                    )

            # ---- stage 2: transpose out1 to o-on-partitions, contract W2
            o1 = spool.tile([B, N * W1W // 4], F32)     # (32, 1024)
            for h in range(2):
                nc.vector.tensor_copy(
                    out=o1[:, h * HB:(h + 1) * HB], in_=ps_h[h][:]
                )
            xT2_ps = ppool.tile([P, NOT * B], F32, space="PSUM", tag="xt2")
            xT2 = spool.tile([P, NOT * B], F32)
            for ot in range(NOT):
                nc.tensor.transpose(
                    out=xT2_ps[:, ot * B:(ot + 1) * B],
                    in_=o1[:, ot * P:(ot + 1) * P],
                    identity=ident[:B, :B],
                )
            nc.vector.tensor_copy(out=xT2[:], in_=xT2_ps[:])

            y_ps = ppool.tile([N, B], F32, space="PSUM", tag="y")
            for ot in range(NOT):
                nc.tensor.matmul(
                    out=y_ps[:],
                    lhsT=w1sb[N - 1][:, W1W + ot * 4:W1W + (ot + 1) * 4],
                    rhs=xT2[:, ot * B:(ot + 1) * B],
                    start=(ot == 0),
                    stop=(ot == NOT - 1),
                )

            ysb = spool.tile([N, B], F32)
            nc.vector.tensor_copy(out=ysb[:], in_=y_ps[:])
            nc.scalar.dma_start(out=out[:], in_=ysb[:])

    nc.finalize()
    return nc


def _get_nc():
    global _nc_cache
    if _nc_cache is None:
        _nc_cache = _build_nc()
    return _nc_cache


def kernel(hidden, pos, W1, b1, W2, b2):
    global LAST_EXEC_NS
    hidden = np.asarray(hidden, dtype=np.float32)
    pos = np.asarray(pos)
    W1 = np.asarray(W1, dtype=np.float32)
    b1 = np.asarray(b1, dtype=np.float32)
    W2 = np.asarray(W2, dtype=np.float32)
    b2 = np.asarray(b2, dtype=np.float32)

    posr = pos.reshape(B, 1).astype(np.int32)

    # W1 (H, N*H) -> per-core [kp, n*1024 + o] = W1[o, n*1024 + j*128 + kp]
    w1r = W1.reshape(H, N, NCORES, HS)                 # [o, n, j, kp]
    # W2 block (replicated): [op, ot*4 + t] = W2[t, ot*128 + op]
    w2blk = np.ascontiguousarray(
        W2.reshape(N, NOT, P).transpose(2, 1, 0).reshape(P, NOT * N)
    ).astype(np.float32)
    # bias fold: c[t] = (W2 @ b1 + b2)[t], added on the host
    cbias = (W2.astype(np.float64) @ b1.astype(np.float64)
             + b2.astype(np.float64))

    in_maps = []
    for j in range(NCORES):
        hid_j = np.ascontiguousarray(
            hidden[:, :, j * HS:(j + 1) * HS]
        ).reshape(B * L, HS)
        w1_j = np.ascontiguousarray(
            w1r[:, :, j, :].transpose(2, 1, 0).reshape(P, N * W1W)
        )                                              # [kp, n*1024+o]
        w1t_j = np.concatenate([w1_j, w2blk], axis=1)  # (128, 4128)
        in_maps.append({"hid": hid_j, "posr": posr, "w1t": w1t_j})

    nc = _get_nc()
    res = run_bass_kernel_spmd(nc, in_maps, list(range(NCORES)), trace=TRACE)
    LAST_EXEC_NS = res.exec_time_ns

    parts = np.stack([res.results[j]["out"] for j in range(NCORES)])  # (8,4,32)
    y = parts.sum(axis=0, dtype=np.float64) + cbias[:, None]          # (4,32)
    return np.ascontiguousarray(y.T.astype(np.float32))               # (B, N)


# revision 15
# speedup vs baseline: 1.2099x; 1.2099x over previous
"""Trainium2 Bass kernel for nn_Expert (gather-span + 2-layer linear MLP).

Reference computation (B=32, L=4096, H=1024, N=4):
    idx      = pos + arange(N)                      # (B, N)
    gathered = hidden[b, idx[b, n], :]              # (B, N, H)
    x        = gathered.reshape(B, N*H)             # (B, 4096)
    out      = (x @ W1.T + b1) @ W2.T + b2          # (B, 4)

Sharding (8 cores): hidden sharded on the LAST dim (H) in 128-wide
slices; W1 sharded over the matching contraction columns (2MB fp32 per
core -- the dominant HBM traffic); pos replicated as an int32 column;
W2 replicated, packed as a 32-col tail on the W1 dram tensor; b1/b2
fold into the host-side constant c = W2 @ b1 + b2 (pure weight prep).

Design notes (from trace analysis):
  * exec time = span minus a ~6us fixed prologue, but the sem-reset
    teardown DOES count and costs ~50-90ns per allocated semaphore --
    keep the instruction/edge count small.
  * fp32 matmul runs at 4 cycles/row; the PE HAM clock gate starts at
    1.2 GHz and reaches 2.4 GHz only after ~3.4us of sustained PE work,
    so zero-dependency junk matmuls warm the array while W1 streams.
  * stage 1 is b-major (xT chunk stationary -- only 32-col LDWEIGHTS;
    W1 streams as the moving operand) in two 512-col PSUM banks; the
    four W1 chunk DMAs land left-to-right so matmuls chase the stream.
  * stage 2 PE-transposes out1 back to o-on-partitions and contracts
    with the packed W2 block: no wide DVE reductions, no replication.
"""

import numpy as np

from concourse import bass, bacc, mybir
from concourse.tile import TileContext
from concourse.bass_utils import run_bass_kernel_spmd
from concourse.masks import make_identity

B, L, H, N = 32, 4096, 1024, 4
NCORES = 8
HS = H // NCORES       # 128: per-core slice of the hidden dim
P = 128
NOT = 8                # o-tiles of 128 in stage 2
HB = 512               # psum bank width (fp32)
W1W = 1024             # free cols per n-chunk of w1
W2OFF = N * W1W        # 4096: w2 block offset in the combined tensor
F32 = mybir.dt.float32
I32 = mybir.dt.int32

WARMUP_MMS = 8         # junk fp32 N=128 matmuls to warm the PE clock

TRACE = False
LAST_EXEC_NS = None

_nc_cache = None


def _build_nc():
    nc = bacc.Bacc(target_bir_lowering=False)
    hid = nc.declare_dram_parameter("hid", [B * L, HS], F32, isOutput=False)
    posr = nc.declare_dram_parameter("posr", [B, 1], I32, isOutput=False)
    w1t = nc.declare_dram_parameter("w1t", [P, W2OFF + 32], F32, isOutput=False)
    out = nc.declare_dram_parameter("out", [N, B], F32, isOutput=True)

    with TileContext(nc) as tc:
        with (
            tc.tile_pool(name="sbuf", bufs=1) as spool,
            tc.tile_pool(name="psum", bufs=1, space="PSUM") as ppool,
        ):
            # ---- pos column first; the W1 stream is gated behind the
            # gather (below), so this completes on quiet queues (HWDGE:
            # ~0.7us vs ~1.5us on the gpsimd SWDGE path)
            pos_col = spool.tile([B, 1], I32)
            nc.scalar.dma_start(out=pos_col[:], in_=posr[:])
            w1sb = []
            for n in range(N):
                w = W1W if n < N - 1 else W1W + 32
                t = spool.tile([P, w], F32, tag=f"w1_{n}", name=f"w1_{n}")
                w1sb.append(t)

            # ---- junk matmuls warm the PE clock gate while W1 streams
            junk = spool.tile([P, P], F32, tag="junk")
            nc.vector.memset(junk[:], 0.0)
            wu_ps = ppool.tile([P, P], F32, space="PSUM", tag="wu")
            for i in range(WARMUP_MMS):
                nc.tensor.matmul(
                    out=wu_ps[:], lhsT=junk[:], rhs=junk[:],
                    start=True, stop=True,
                )

            # ---- gather indices: idx[b] = b*L + pos[b]
            gci = spool.tile([B, 1], I32)
            nc.gpsimd.iota(gci[:], pattern=[[0, 1]], base=0,
                           channel_multiplier=L)
            idx = spool.tile([B, 1], I32)
            nc.gpsimd.tensor_tensor(
                out=idx[:], in0=gci[:], in1=pos_col[:],
                op=mybir.AluOpType.add,
            )

            # ---- gather: 32 spans of 4 consecutive hid rows (2KB each)
            xg = spool.tile([B, N * HS], F32)
            nc.gpsimd.indirect_dma_start(
                out=xg[:, :],
                out_offset=None,
                in_=hid[:],
                in_offset=bass.IndirectOffsetOnAxis(ap=idx[:, :1], axis=0),
                bounds_check=None,
            )

            # ---- W1 chunks, gated behind xg: a stub write into each chunk
            # tile that reads xg forces the chunk DMA (WAR) to wait until
            # the gather has drained -- tiny DMAs starve for ~3.5us when
            # the 2MB W1 stream floods the 16 SDMA engines first.
            for n in range(N):
                nc.vector.tensor_copy(out=w1sb[n][:1, :1], in_=xg[:1, :1])
                eng = nc.scalar if n % 2 == 0 else nc.sync
                eng.dma_start(
                    out=w1sb[n][:],
                    in_=w1t[:, n * W1W:n * W1W + (W1W if n < N - 1 else W1W + 32)],
                )

            # ---- transpose each 128-col chunk: xT[:, n*32+b] = x[b,(n,hp)]
            ident = spool.tile([P, P], F32)
            make_identity(nc, ident[:])
            xT_ps = ppool.tile([P, P], F32, space="PSUM", tag="xt")
            xT = spool.tile([P, P], F32)
            for n in range(N):
                nc.tensor.transpose(
                    out=xT_ps[:, n * B:(n + 1) * B],
                    in_=xg[:, n * HS:(n + 1) * HS],
                    identity=ident[:B, :B],
                )
            nc.vector.tensor_copy(out=xT[:], in_=xT_ps[:])

            # ---- stage 1 b-major: out1[b, o] in two 512-col banks
            ps_h = [
                ppool.tile([B, HB], F32, space="PSUM", tag=f"ps{h}",
                           name=f"ps{h}")
                for h in range(2)
            ]
            for n in range(N):
                for h in range(2):
                    nc.tensor.matmul(
                        out=ps_h[h][:],
                        lhsT=xT[:, n * B:(n + 1) * B],
                        rhs=w1sb[n][:, h * HB:(h + 1) * HB],
                        start=(n == 0),
                        stop=(n == N - 1),
                    )

            # ---- stage 2: transpose out1 to o-on-partitions, contract W2
            o1 = spool.tile([B, N * W1W // 4], F32)     # (32, 1024)
            for h in range(2):
                nc.vector.tensor_copy(
                    out=o1[:, h * HB:(h + 1) * HB], in_=ps_h[h][:]
                )
            xT2_ps = ppool.tile([P, NOT * B], F32, space="PSUM", tag="xt2")
            xT2 = spool.tile([P, NOT * B], F32)
            for ot in range(NOT):
                nc.tensor.transpose(
                    out=xT2_ps[:, ot * B:(ot + 1) * B],
                    in_=o1[:, ot * P:(ot + 1) * P],
                    identity=ident[:B, :B],
                )
            nc.vector.tensor_copy(out=xT2[:], in_=xT2_ps[:])

            y_ps = ppool.tile([N, B], F32, space="PSUM", tag="y")
            for ot in range(NOT):
                nc.tensor.matmul(
                    out=y_ps[:],
                    lhsT=w1sb[N - 1][:, W1W + ot * 4:W1W + (ot + 1) * 4],
                    rhs=xT2[:, ot * B:(ot + 1) * B],
                    start=(ot == 0),
                    stop=(ot == NOT - 1),
                )

            ysb = spool.tile([N, B], F32)
            nc.vector.tensor_copy(out=ysb[:], in_=y_ps[:])
            nc.scalar.dma_start(out=out[:], in_=ysb[:])

    nc.finalize()
    return nc


def _get_nc():
    global _nc_cache
    if _nc_cache is None:
        _nc_cache = _build_nc()
    return _nc_cache


def kernel(hidden, pos, W1, b1, W2, b2):
    global LAST_EXEC_NS
    hidden = np.asarray(hidden, dtype=np.float32)
    pos = np.asarray(pos)
    W1 = np.asarray(W1, dtype=np.float32)
    b1 = np.asarray(b1, dtype=np.float32)
    W2 = np.asarray(W2, dtype=np.float32)
    b2 = np.asarray(b2, dtype=np.float32)

    posr = pos.reshape(B, 1).astype(np.int32)

    # W1 (H, N*H) -> per-core [kp, n*1024 + o] = W1[o, n*1024 + j*128 + kp]
    w1r = W1.reshape(H, N, NCORES, HS)                 # [o, n, j, kp]
    # W2 block (replicated): [op, ot*4 + t] = W2[t, ot*128 + op]
    w2blk = np.ascontiguousarray(
        W2.reshape(N, NOT, P).transpose(2, 1, 0).reshape(P, NOT * N)
    ).astype(np.float32)
    # bias fold: c[t] = (W2 @ b1 + b2)[t], added on the host
    cbias = (W2.astype(np.float64) @ b1.astype(np.float64)
             + b2.astype(np.float64))

    in_maps = []
    for j in range(NCORES):
        hid_j = np.ascontiguousarray(
            hidden[:, :, j * HS:(j + 1) * HS]
        ).reshape(B * L, HS)
        w1_j = np.ascontiguousarray(
            w1r[:, :, j, :].transpose(2, 1, 0).reshape(P, N * W1W)
        )                                              # [kp, n*1024+o]
        w1t_j = np.concatenate([w1_j, w2blk], axis=1)  # (128, 4128)
        in_maps.append({"hid": hid_j, "posr": posr, "w1t": w1t_j})

    nc = _get_nc()
    res = run_bass_kernel_spmd(nc, in_maps, list(range(NCORES)), trace=TRACE)
    LAST_EXEC_NS = res.exec_time_ns

    parts = np.stack([res.results[j]["out"] for j in range(NCORES)])  # (8,4,32)
    y = parts.sum(axis=0, dtype=np.float64) + cbias[:, None]          # (4,32)
    return np.ascontiguousarray(y.T.astype(np.float32))               # (B, N)
